# revision 14
# baseline (speedup 1.0000x reference)
"""CenterLossLayer kernel for 8 Trainium2 NeuronCores (raw Bass).

Model-parallel over the class dimension (nrof_classes = 50000, padded to
50176 = 8 * 6272). Each core owns a contiguous class range and:
  - finds, for every batch row, the local label (position of the 1 in its
    one-hot shard) via iota-multiply + max-reduce on the Vector engine,
  - gathers the selected center rows with an indirect DMA,
  - resolves duplicate labels with 128x128 is_equal selection matrices and
    PE matmul group-sums,
  - bulk-copies its centers shard to the output (DRAM->DRAM DMA) and
    scatters the <=256 updated rows on top (indirect DMA; rows whose label
    is not in this shard get index ~BIG and are skipped via bounds_check),
  - emits a masked partial of the squared-distance result.

Host side only pads/shards inputs and concatenates/sums outputs.

new_centers[c] = centers[c] - ALPHA/(cnt_c+1) * sum_{b:label_b=c}(centers[c]-x_b)
result[b]      = ||x_b - centers[label_b]||^2
"""

import numpy as np

import concourse.bass as bass
import concourse.mybir as mybir
from concourse.bass_utils import run_bass_kernel_spmd

B, C, D = 256, 50000, 512
ALPHA = 0.5
NCORES = 8
P = 128
CPAD = 50176                 # 8 * 49 * 128
KC = CPAD // NCORES          # 6272 classes per core
BT = B // P                  # 2 batch tiles
BIG = 100000.0               # sentinel offset for "label not in my shard"
F32 = mybir.dt.float32
I32 = mybir.dt.int32
Alu = mybir.AluOpType

_cached_nc = None


def build_bass(debug_taps=False):
    # detect_race_conditions=False: the sim's race detector does not credit
    # same-engine program order (HW serializes in-order per engine pipeline);
    # cross-engine ordering is fully covered by explicit semaphores below.
    nc = bass.Bass(
        "TRN2", target_bir_lowering=False, debug=False,
        detect_race_conditions=False,
    )

    x_t = nc.dram_tensor("x", [B, D], F32, kind="ExternalInput")
    oh_t = nc.dram_tensor("oh", [B, KC], F32, kind="ExternalInput")
    cen_t = nc.dram_tensor("cen", [KC, D], F32, kind="ExternalInput")
    ncen_t = nc.dram_tensor("ncen", [KC, D], F32, kind="ExternalOutput")
    res_t = nc.dram_tensor("res", [B, 1], F32, kind="ExternalOutput")

    taps = {}
    if debug_taps:
        for name, shape, dt in [
            ("t_lp0", [P, 1], F32), ("t_m0", [P, 1], F32),
            ("t_gidx0", [P, 1], I32), ("t_sk0", [P, 1], F32),
            ("t_ski0", [P, 1], I32), ("t_cnt0", [P, 1], F32),
            ("t_scl0", [P, 1], F32), ("t_iota", [P, 16], F32),
            ("t_ident", [P, P], F32), ("t_skT0", [P, P], F32),
            ("t_eq00", [P, P], F32), ("t_g0", [P, D], F32),
            ("t_d0", [P, D], F32), ("t_ssc0", [P, D], F32),
            ("t_upd0", [P, D], F32),
        ]:
            taps[name] = nc.dram_tensor(name, shape, dt, kind="ExternalOutput")

    from contextlib import ExitStack

    with ExitStack() as ctx:
        ent = ctx.enter_context
        oh0 = ent(nc.sbuf_tensor([P, KC], F32))
        oh1 = ent(nc.sbuf_tensor([P, KC], F32))
        iota = ent(nc.sbuf_tensor([P, KC], F32))
        ident = ent(nc.sbuf_tensor([P, P], F32))
        x0 = ent(nc.sbuf_tensor([P, D], F32))
        x1 = ent(nc.sbuf_tensor([P, D], F32))
        g0 = ent(nc.sbuf_tensor([P, D], F32))
        g1 = ent(nc.sbuf_tensor([P, D], F32))
        d0 = ent(nc.sbuf_tensor([P, D], F32))
        d1 = ent(nc.sbuf_tensor([P, D], F32))
        d2j = ent(nc.sbuf_tensor([P, D], F32))
        ssc0 = ent(nc.sbuf_tensor([P, D], F32))
        ssc1 = ent(nc.sbuf_tensor([P, D], F32))
        upd0 = ent(nc.sbuf_tensor([P, D], F32))
        upd1 = ent(nc.sbuf_tensor([P, D], F32))
        skT0 = ent(nc.sbuf_tensor([P, P], F32))
        skT1 = ent(nc.sbuf_tensor([P, P], F32))
        eq00 = ent(nc.sbuf_tensor([P, P], F32))
        eq01 = ent(nc.sbuf_tensor([P, P], F32))
        eq10 = ent(nc.sbuf_tensor([P, P], F32))
        eq11 = ent(nc.sbuf_tensor([P, P], F32))
        _small_names = [
            "lp0", "lp1", "m0", "m1", "idxf0", "idxf1", "gidxf0", "gidxf1",
            "tb0", "tb1", "sk0", "sk1", "c00", "c01", "c10", "c11", "cnt0",
            "cnt1", "rec0", "rec1", "scl0", "scl1", "r00", "r01", "rr0", "rr1",
        ]
        _small = {
            n: ent(nc.sbuf_tensor(n, [P, 1], F32)) for n in _small_names
        }
        (lp0, lp1, m0, m1, idxf0, idxf1, gidxf0, gidxf1, tb0, tb1, sk0, sk1,
         c00, c01, c10, c11, cnt0, cnt1, rec0, rec1, scl0, scl1, r00, r01,
         rr0, rr1) = (_small[n] for n in _small_names)
        gidx0 = ent(nc.sbuf_tensor([P, 1], I32))
        gidx1 = ent(nc.sbuf_tensor([P, 1], I32))
        ski0 = ent(nc.sbuf_tensor([P, 1], I32))
        ski1 = ent(nc.sbuf_tensor([P, 1], I32))
        ps0 = ent(nc.psum_tensor([P, P], F32))
        ps1 = ent(nc.psum_tensor([P, P], F32))
        S0 = ent(nc.psum_tensor([P, D], F32))
        S1 = ent(nc.psum_tensor([P, D], F32))
        s_oh = ent(nc.semaphore("s_oh"))
        s_x = ent(nc.semaphore("s_x"))
        s_cp = ent(nc.semaphore("s_cp"))
        s_g = ent(nc.semaphore("s_g"))
        s_sc = ent(nc.semaphore("s_sc"))
        s_res = ent(nc.semaphore("s_res"))
        s_v = ent(nc.semaphore("s_v"))
        s_pe = ent(nc.semaphore("s_pe"))
        s_gp = ent(nc.semaphore("s_gp"))
        block = ent(nc.Block())
        # Milestones on the (strictly ordered) vector stream. The vector
        # block is emitted first so consumers can wait on concrete counts.
        #
        # TRN2 DVE does NOT interlock back-to-back dependent ops: a short op's
        # writes are still in the 8-slice pipe when the next op reads
        # (HW-verified). Every DVE op is therefore followed by an explicit
        # drain, and cross-engine milestones increment on the drain so data
        # is published before consumers wake.
        vm = {"n": 0}

        def vdrain(name=None):
            d = nc.vector.drain()
            if name is not None:
                d.then_inc(s_v, 1)
                vm["n"] += 1
                vm[name] = vm["n"]

        @block.vector
        def _(vector):
            for t, (oh_s, lp, m, idxf, gidxf, gidx, tb, sk, ski) in enumerate([
                (oh0, lp0, m0, idxf0, gidxf0, gidx0, tb0, sk0, ski0),
                (oh1, lp1, m1, idxf1, gidxf1, gidx1, tb1, sk1, ski1),
            ]):
                if t == 0:
                    # DMAs on one sem can complete out of order; wait for
                    # both oh tiles before touching either.
                    nc.vector.wait_ge(s_oh, 32)
                    nc.vector.wait_ge(s_gp, 1)
                nc.vector.tensor_tensor(
                    out=oh_s[:], in0=oh_s[:], in1=iota[:], op=Alu.mult
                )
                vdrain()
                nc.vector.tensor_reduce(
                    out=lp[:], in_=oh_s[:], axis=mybir.AxisListType.X, op=Alu.max
                )
                vdrain()
                nc.vector.tensor_scalar(
                    out=m[:], in0=lp[:], scalar1=0.0, scalar2=None, op0=Alu.is_gt
                )
                vdrain()
                nc.vector.tensor_scalar_add(idxf[:], lp[:], -1.0)
                vdrain()
                nc.vector.tensor_scalar_max(gidxf[:], idxf[:], 0.0)
                vdrain()
                nc.vector.tensor_copy(gidx[:], gidxf[:])
                vdrain(f"V_GIDX{t}")
                nc.vector.tensor_scalar(
                    out=tb[:], in0=m[:], scalar1=-BIG, scalar2=BIG,
                    op0=Alu.mult, op1=Alu.add,
                )
                vdrain()
                nc.vector.tensor_tensor(
                    out=sk[:], in0=idxf[:], in1=tb[:], op=Alu.add
                )
                vdrain(f"V_SKEY{t}")
                nc.vector.tensor_copy(ski[:], sk[:])
                vdrain()

            # selection matrices eq[i][j][p, q] = (skey_i[p] == skey_j[q])
            nc.vector.wait_ge(s_pe, 1)
            nc.vector.tensor_copy(skT0[:], ps0[:])
            vdrain()
            nc.vector.wait_ge(s_pe, 2)
            nc.vector.tensor_copy(skT1[:], ps1[:])
            vdrain()
            nc.vector.tensor_tensor(
                out=eq00[:], in0=sk0[:].to_broadcast([P, P]), in1=skT0[:],
                op=Alu.is_equal,
            )
            vdrain()
            nc.vector.tensor_tensor(
                out=eq01[:], in0=sk0[:].to_broadcast([P, P]), in1=skT1[:],
                op=Alu.is_equal,
            )
            vdrain()
            nc.vector.tensor_tensor(
                out=eq10[:], in0=sk1[:].to_broadcast([P, P]), in1=skT0[:],
                op=Alu.is_equal,
            )
            vdrain()
            nc.vector.tensor_tensor(
                out=eq11[:], in0=sk1[:].to_broadcast([P, P]), in1=skT1[:],
                op=Alu.is_equal,
            )
            vdrain()

            # d = gathered - x  (wait for both gathers/loads: completion of
            # two DMAs on one sem is unordered)
            nc.vector.wait_ge(s_g, 32)
            nc.vector.wait_ge(s_x, 32)
            nc.vector.tensor_tensor(out=d0[:], in0=g0[:], in1=x0[:], op=Alu.subtract)
            vdrain()
            nc.vector.tensor_tensor(out=d1[:], in0=g1[:], in1=x1[:], op=Alu.subtract)
            vdrain("V_EQD")

            # result partials: m * sum(d^2)
            nc.vector.tensor_tensor(out=d2j[:], in0=d0[:], in1=d0[:], op=Alu.mult)
            vdrain()
            nc.vector.tensor_reduce(
                out=r00[:], in_=d2j[:], axis=mybir.AxisListType.X, op=Alu.add
            )
            vdrain()
            nc.vector.tensor_tensor(out=rr0[:], in0=r00[:], in1=m0[:], op=Alu.mult)
            vdrain("V_RR0")
            nc.vector.tensor_tensor(out=d2j[:], in0=d1[:], in1=d1[:], op=Alu.mult)
            vdrain()
            nc.vector.tensor_reduce(
                out=r01[:], in_=d2j[:], axis=mybir.AxisListType.X, op=Alu.add
            )
            vdrain()
            nc.vector.tensor_tensor(out=rr1[:], in0=r01[:], in1=m1[:], op=Alu.mult)
            vdrain("V_RR1")

            # counts and scales: scl = ALPHA / (cnt + 1)
            nc.vector.reduce_sum(c00[:], eq00[:], axis=mybir.AxisListType.X)
            vdrain()
            nc.vector.reduce_sum(c01[:], eq01[:], axis=mybir.AxisListType.X)
            vdrain()
            nc.vector.tensor_tensor(out=cnt0[:], in0=c00[:], in1=c01[:], op=Alu.add)
            vdrain()
            nc.vector.tensor_scalar_add(cnt0[:], cnt0[:], 1.0)
            vdrain()
            nc.vector.reciprocal(rec0[:], cnt0[:])
            vdrain()
            nc.vector.tensor_scalar_mul(scl0[:], rec0[:], ALPHA)
            vdrain()
            nc.vector.reduce_sum(c10[:], eq10[:], axis=mybir.AxisListType.X)
            vdrain()
            nc.vector.reduce_sum(c11[:], eq11[:], axis=mybir.AxisListType.X)
            vdrain()
            nc.vector.tensor_tensor(out=cnt1[:], in0=c10[:], in1=c11[:], op=Alu.add)
            vdrain()
            nc.vector.tensor_scalar_add(cnt1[:], cnt1[:], 1.0)
            vdrain()
            nc.vector.reciprocal(rec1[:], cnt1[:])
            vdrain()
            nc.vector.tensor_scalar_mul(scl1[:], rec1[:], ALPHA)
            vdrain()

            # updated rows: upd = g - scl * S
            nc.vector.wait_ge(s_pe, 3)
            nc.vector.tensor_scalar(
                out=ssc0[:], in0=S0[:], scalar1=scl0[:, :1], scalar2=None,
                op0=Alu.mult,
            )
            vdrain()
            nc.vector.tensor_tensor(
                out=upd0[:], in0=g0[:], in1=ssc0[:], op=Alu.subtract
            )
            vdrain("V_UPD0")
            nc.vector.wait_ge(s_pe, 4)
            nc.vector.tensor_scalar(
                out=ssc1[:], in0=S1[:], scalar1=scl1[:, :1], scalar2=None,
                op0=Alu.mult,
            )
            vdrain()
            nc.vector.tensor_tensor(
                out=upd1[:], in0=g1[:], in1=ssc1[:], op=Alu.subtract
            )
            vdrain("V_UPD1")

        @block.tensor
        def _(tensor):
            nc.tensor.wait_ge(s_gp, 2)
            nc.tensor.wait_ge(s_v, vm["V_SKEY0"])
            nc.tensor.transpose(
                out=ps0[:], in_=sk0[:].to_broadcast([P, P]), identity=ident[:]
            ).then_inc(s_pe, 1)
            nc.tensor.wait_ge(s_v, vm["V_SKEY1"])
            nc.tensor.transpose(
                out=ps1[:], in_=sk1[:].to_broadcast([P, P]), identity=ident[:]
            ).then_inc(s_pe, 1)

            nc.tensor.wait_ge(s_v, vm["V_EQD"])
            nc.tensor.matmul(out=S0[:], lhsT=eq00[:], rhs=d0[:], start=True, stop=False)
            nc.tensor.matmul(
                out=S0[:], lhsT=eq10[:], rhs=d1[:], start=False, stop=True
            ).then_inc(s_pe, 1)
            nc.tensor.matmul(out=S1[:], lhsT=eq01[:], rhs=d0[:], start=True, stop=False)
            nc.tensor.matmul(
                out=S1[:], lhsT=eq11[:], rhs=d1[:], start=False, stop=True
            ).then_inc(s_pe, 1)

        @block.gpsimd
        def _(gpsimd):
            nc.gpsimd.iota(
                iota[:],
                pattern=[[1, KC]],
                base=1,
                channel_multiplier=0,
                allow_small_or_imprecise_dtypes=True,
            ).then_inc(s_gp, 1)
            # identity matrix (inlined make_identity so we can chain the inc);
            # gpsimd ops can overlap across Q7 cores, so drain between the
            # memset and the affine_select that reads it.
            nc.gpsimd.memset(ident[:], 0.0)
            nc.gpsimd.drain()
            nc.gpsimd.affine_select(
                out=ident[:],
                in_=ident[:],
                compare_op=Alu.not_equal,
                fill=1.0,
                base=0,
                pattern=[[-1, P]],
                channel_multiplier=1,
            ).then_inc(s_gp, 1)

            nc.gpsimd.wait_ge(s_v, vm["V_GIDX0"])
            nc.gpsimd.indirect_dma_start(
                out=g0[:], out_offset=None, in_=cen_t[:],
                in_offset=bass.IndirectOffsetOnAxis(ap=gidx0[:, :1], axis=0),
            ).then_inc(s_g, 16)
            nc.gpsimd.wait_ge(s_v, vm["V_GIDX1"])
            nc.gpsimd.indirect_dma_start(
                out=g1[:], out_offset=None, in_=cen_t[:],
                in_offset=bass.IndirectOffsetOnAxis(ap=gidx1[:, :1], axis=0),
            ).then_inc(s_g, 16)

            nc.gpsimd.wait_ge(s_cp, 16)
            nc.gpsimd.wait_ge(s_v, vm["V_UPD0"])
            nc.gpsimd.indirect_dma_start(
                out=ncen_t[:],
                out_offset=bass.IndirectOffsetOnAxis(ap=ski0[:, :1], axis=0),
                in_=upd0[:], in_offset=None,
                bounds_check=KC - 1, oob_is_err=False,
            ).then_inc(s_sc, 16)
            nc.gpsimd.wait_ge(s_v, vm["V_UPD1"])
            nc.gpsimd.indirect_dma_start(
                out=ncen_t[:],
                out_offset=bass.IndirectOffsetOnAxis(ap=ski1[:, :1], axis=0),
                in_=upd1[:], in_offset=None,
                bounds_check=KC - 1, oob_is_err=False,
            ).then_inc(s_sc, 16)

        @block.scalar
        def _(scalar):
            # Bulk copy of the (untouched) centers shard, DRAM -> DRAM, on
            # the ACT HWDGE ring so it doesn't queue behind the SP loads.
            nc.scalar.dma_start(out=ncen_t[:], in_=cen_t[:]).then_inc(s_cp, 16)

        @block.sync
        def _(sync):
            nc.sync.dma_start(out=oh0[:], in_=oh_t[0:P, :]).then_inc(s_oh, 16)
            nc.sync.dma_start(out=oh1[:], in_=oh_t[P:B, :]).then_inc(s_oh, 16)
            nc.sync.dma_start(out=x0[:], in_=x_t[0:P, :]).then_inc(s_x, 16)
            nc.sync.dma_start(out=x1[:], in_=x_t[P:B, :]).then_inc(s_x, 16)
            nc.sync.wait_ge(s_v, vm["V_RR0"])
            nc.sync.dma_start(out=res_t[0:P, :], in_=rr0[:]).then_inc(s_res, 16)
            nc.sync.wait_ge(s_v, vm["V_RR1"])
            nc.sync.dma_start(out=res_t[P:B, :], in_=rr1[:]).then_inc(s_res, 16)
            # Kernel-exit safety: everything observable has landed.
            nc.sync.wait_ge(s_sc, 32)
            nc.sync.wait_ge(s_res, 32)
            if debug_taps:
                nc.sync.wait_ge(s_v, vm["n"])
                nc.sync.wait_ge(s_pe, 4)
                nc.sync.wait_ge(s_g, 32)
                src = {
                    "t_lp0": lp0, "t_m0": m0, "t_gidx0": gidx0,
                    "t_sk0": sk0, "t_ski0": ski0, "t_cnt0": cnt0,
                    "t_scl0": scl0, "t_ident": ident, "t_skT0": skT0,
                    "t_eq00": eq00, "t_g0": g0, "t_d0": d0,
                    "t_ssc0": ssc0, "t_upd0": upd0,
                }
                for name, buf in src.items():
                    nc.sync.dma_start(
                        out=taps[name][:], in_=buf[:]
                    ).then_inc(s_res, 16)
                nc.sync.dma_start(
                    out=taps["t_iota"][:], in_=iota[:, 0:16]
                ).then_inc(s_res, 16)
                nc.sync.wait_ge(s_res, 32 + 16 * 15)

    return nc


def _get_nc():
    global _cached_nc
    if _cached_nc is None:
        _cached_nc = build_bass()
    return _cached_nc


def kernel(x, onehot, centers, _debug_results=None):
    x = np.ascontiguousarray(np.asarray(x, dtype=np.float32))
    onehot = np.asarray(onehot, dtype=np.float32)
    centers = np.asarray(centers, dtype=np.float32)

    oh_pad = np.zeros((B, CPAD), dtype=np.float32)
    oh_pad[:, :C] = onehot
    cen_pad = np.zeros((CPAD, D), dtype=np.float32)
    cen_pad[:C] = centers

    in_maps = []
    for i in range(NCORES):
        sl = slice(i * KC, (i + 1) * KC)
        in_maps.append({
            "x": x,
            "oh": np.ascontiguousarray(oh_pad[:, sl]),
            "cen": np.ascontiguousarray(cen_pad[sl]),
        })

    nc = _get_nc()
    br = run_bass_kernel_spmd(nc, in_maps, core_ids=list(range(NCORES)))
    outs = br.results
    if _debug_results is not None:
        _debug_results.append(br)

    new_centers = np.concatenate([o["ncen"] for o in outs], axis=0)[:C]
    result = np.zeros((B, 1), dtype=np.float32)
    for o in outs:
        result = result + o["res"]
    return result, new_centers


# revision 16
# speedup vs baseline: 31050.8405x; 31050.8405x over previous
"""CenterLossLayer kernel for 8 Trainium2 NeuronCores (raw Bass).

Model-parallel over the class dimension (nrof_classes = 50000, padded to
50176 = 8 * 6272). Each core owns a contiguous class range and:
  - finds, for every batch row, the local label (position of the 1 in its
    one-hot shard) via iota-multiply + max-reduce on the Vector engine,
  - gathers the selected center rows with an indirect DMA,
  - resolves duplicate labels with 128x128 is_equal selection matrices and
    PE matmul group-sums,
  - bulk-copies its centers shard to the output (DRAM->DRAM DMA) and
    scatters the <=256 updated rows on top (indirect DMA; rows whose label
    is not in this shard get index ~BIG and are skipped via bounds_check),
  - emits a masked partial of the squared-distance result.

Host side only pads/shards inputs and concatenates/sums outputs.

new_centers[c] = centers[c] - ALPHA/(cnt_c+1) * sum_{b:label_b=c}(centers[c]-x_b)
result[b]      = ||x_b - centers[label_b]||^2
"""

import numpy as np

import concourse.bass as bass
import concourse.mybir as mybir
from concourse.bass_utils import run_bass_kernel_spmd

B, C, D = 256, 50000, 512
ALPHA = 0.5
NCORES = 8
P = 128
CPAD = 50176                 # 8 * 49 * 128
KC = CPAD // NCORES          # 6272 classes per core
BT = B // P                  # 2 batch tiles
BIG = 100000.0               # sentinel offset for "label not in my shard"
F32 = mybir.dt.float32
I32 = mybir.dt.int32
Alu = mybir.AluOpType

_cached_nc = None


def build_bass(debug_taps=False, reps=1):
    # detect_race_conditions=False: the sim's race detector does not credit
    # same-engine program order (HW serializes in-order per engine pipeline);
    # cross-engine ordering is fully covered by explicit semaphores below.
    nc = bass.Bass(
        "TRN2", target_bir_lowering=False, debug=False,
        detect_race_conditions=False,
    )

    x_t = nc.dram_tensor("x", [B, D], F32, kind="ExternalInput")
    oh_t = nc.dram_tensor("oh", [B, KC], F32, kind="ExternalInput")
    cen_t = nc.dram_tensor("cen", [KC, D], F32, kind="ExternalInput")
    ncen_t = nc.dram_tensor("ncen", [KC, D], F32, kind="ExternalOutput")
    res_t = nc.dram_tensor("res", [B, 1], F32, kind="ExternalOutput")

    taps = {}
    if debug_taps:
        for name, shape, dt in [
            ("t_lp0", [P, 1], F32), ("t_m0", [P, 1], F32),
            ("t_gidx0", [P, 1], I32), ("t_sk0", [P, 1], F32),
            ("t_ski0", [P, 1], I32), ("t_cnt0", [P, 1], F32),
            ("t_scl0", [P, 1], F32), ("t_iota", [P, 16], F32),
            ("t_ident", [P, P], F32), ("t_skT0", [P, P], F32),
            ("t_eq00", [P, P], F32), ("t_g0", [P, D], F32),
            ("t_d0", [P, D], F32), ("t_ssc0", [P, D], F32),
            ("t_upd0", [P, D], F32),
        ]:
            taps[name] = nc.dram_tensor(name, shape, dt, kind="ExternalOutput")

    from contextlib import ExitStack

    with ExitStack() as ctx:
        ent = ctx.enter_context
        oh0 = ent(nc.sbuf_tensor([P, KC], F32))
        oh1 = ent(nc.sbuf_tensor([P, KC], F32))
        iota = ent(nc.sbuf_tensor([P, KC], F32))
        ident = ent(nc.sbuf_tensor([P, P], F32))
        x0 = ent(nc.sbuf_tensor([P, D], F32))
        x1 = ent(nc.sbuf_tensor([P, D], F32))
        g0 = ent(nc.sbuf_tensor([P, D], F32))
        g1 = ent(nc.sbuf_tensor([P, D], F32))
        d0 = ent(nc.sbuf_tensor([P, D], F32))
        d1 = ent(nc.sbuf_tensor([P, D], F32))
        d2j = ent(nc.sbuf_tensor([P, D], F32))
        ssc0 = ent(nc.sbuf_tensor([P, D], F32))
        ssc1 = ent(nc.sbuf_tensor([P, D], F32))
        upd0 = ent(nc.sbuf_tensor([P, D], F32))
        upd1 = ent(nc.sbuf_tensor([P, D], F32))
        skT0 = ent(nc.sbuf_tensor([P, P], F32))
        skT1 = ent(nc.sbuf_tensor([P, P], F32))
        eq00 = ent(nc.sbuf_tensor([P, P], F32))
        eq01 = ent(nc.sbuf_tensor([P, P], F32))
        eq10 = ent(nc.sbuf_tensor([P, P], F32))
        eq11 = ent(nc.sbuf_tensor([P, P], F32))
        _small_names = [
            "lp0", "lp1", "m0", "m1", "idxf0", "idxf1", "gidxf0", "gidxf1",
            "tb0", "tb1", "sk0", "sk1", "c00", "c01", "c10", "c11", "cnt0",
            "cnt1", "rec0", "rec1", "scl0", "scl1", "r00", "r01", "rr0", "rr1",
        ]
        _small = {
            n: ent(nc.sbuf_tensor(n, [P, 1], F32)) for n in _small_names
        }
        (lp0, lp1, m0, m1, idxf0, idxf1, gidxf0, gidxf1, tb0, tb1, sk0, sk1,
         c00, c01, c10, c11, cnt0, cnt1, rec0, rec1, scl0, scl1, r00, r01,
         rr0, rr1) = (_small[n] for n in _small_names)
        gidx0 = ent(nc.sbuf_tensor([P, 1], I32))
        gidx1 = ent(nc.sbuf_tensor([P, 1], I32))
        ski0 = ent(nc.sbuf_tensor([P, 1], I32))
        ski1 = ent(nc.sbuf_tensor([P, 1], I32))
        ps0 = ent(nc.psum_tensor([P, P], F32))
        ps1 = ent(nc.psum_tensor([P, P], F32))
        S0 = ent(nc.psum_tensor([P, D], F32))
        S1 = ent(nc.psum_tensor([P, D], F32))
        s_oh = ent(nc.semaphore("s_oh"))
        s_x = ent(nc.semaphore("s_x"))
        s_cp = ent(nc.semaphore("s_cp"))
        s_g = ent(nc.semaphore("s_g"))
        s_sc = ent(nc.semaphore("s_sc"))
        s_res = ent(nc.semaphore("s_res"))
        s_v = ent(nc.semaphore("s_v"))
        s_pe = ent(nc.semaphore("s_pe"))
        s_gp = ent(nc.semaphore("s_gp"))
        block = ent(nc.Block())
        # Milestones on the (strictly ordered) vector stream. The vector
        # block is emitted first so consumers can wait on concrete counts.
        #
        # TRN2 DVE does NOT interlock back-to-back dependent ops: a short op's
        # writes are still in the 8-slice pipe when the next op reads
        # (HW-verified). Every DVE op is therefore followed by an explicit
        # drain, and cross-engine milestones increment on the drain so data
        # is published before consumers wake.
        vm = {"n": 0}

        def vdrain(name=None):
            d = nc.vector.drain()
            if name is not None:
                d.then_inc(s_v, 1)
                vm["n"] += 1
                vm[name] = vm["n"]

        REPS = reps

        @block.vector
        def _(vector):
            for r in range(REPS):
                if r > 0:
                    # rep barrier: prior rep's res stores and scatters done
                    # before rr/upd buffers are rewritten.
                    nc.vector.wait_ge(s_res, 32 * r)
                    nc.vector.wait_ge(s_sc, 32 * r)
                for t, (oh_s, lp, m, idxf, gidxf, gidx, tb, sk, ski) in enumerate([
                    (oh0, lp0, m0, idxf0, gidxf0, gidx0, tb0, sk0, ski0),
                    (oh1, lp1, m1, idxf1, gidxf1, gidx1, tb1, sk1, ski1),
                ]):
                    if t == 0:
                        # DMAs on one sem can complete out of order; wait for
                        # both oh tiles before touching either.
                        nc.vector.wait_ge(s_oh, 32 * (r + 1))
                        nc.vector.wait_ge(s_gp, 1)
                    nc.vector.tensor_tensor(
                        out=oh_s[:], in0=oh_s[:], in1=iota[:], op=Alu.mult
                    )
                    vdrain()
                    nc.vector.tensor_reduce(
                        out=lp[:], in_=oh_s[:], axis=mybir.AxisListType.X, op=Alu.max
                    )
                    vdrain()
                    nc.vector.tensor_scalar(
                        out=m[:], in0=lp[:], scalar1=0.0, scalar2=None, op0=Alu.is_gt
                    )
                    vdrain()
                    nc.vector.tensor_scalar_add(idxf[:], lp[:], -1.0)
                    vdrain()
                    nc.vector.tensor_scalar_max(gidxf[:], idxf[:], 0.0)
                    vdrain()
                    nc.vector.tensor_copy(gidx[:], gidxf[:])
                    vdrain(f"V_GIDX{t}_r{r}")
                    nc.vector.tensor_scalar(
                        out=tb[:], in0=m[:], scalar1=-BIG, scalar2=BIG,
                        op0=Alu.mult, op1=Alu.add,
                    )
                    vdrain()
                    nc.vector.tensor_tensor(
                        out=sk[:], in0=idxf[:], in1=tb[:], op=Alu.add
                    )
                    vdrain(f"V_SKEY{t}_r{r}")
                    nc.vector.tensor_copy(ski[:], sk[:])
                    vdrain()

                # selection matrices eq[i][j][p, q] = (skey_i[p] == skey_j[q])
                nc.vector.wait_ge(s_pe, 4 * r + 1)
                nc.vector.tensor_copy(skT0[:], ps0[:])
                vdrain()
                nc.vector.wait_ge(s_pe, 4 * r + 2)
                nc.vector.tensor_copy(skT1[:], ps1[:])
                vdrain()
                nc.vector.tensor_tensor(
                    out=eq00[:], in0=sk0[:].to_broadcast([P, P]), in1=skT0[:],
                    op=Alu.is_equal,
                )
                vdrain()
                nc.vector.tensor_tensor(
                    out=eq01[:], in0=sk0[:].to_broadcast([P, P]), in1=skT1[:],
                    op=Alu.is_equal,
                )
                vdrain()
                nc.vector.tensor_tensor(
                    out=eq10[:], in0=sk1[:].to_broadcast([P, P]), in1=skT0[:],
                    op=Alu.is_equal,
                )
                vdrain()
                nc.vector.tensor_tensor(
                    out=eq11[:], in0=sk1[:].to_broadcast([P, P]), in1=skT1[:],
                    op=Alu.is_equal,
                )
                vdrain()

                # d = gathered - x  (wait for both gathers/loads: completion
                # of two DMAs on one sem is unordered)
                nc.vector.wait_ge(s_g, 32 * (r + 1))
                nc.vector.wait_ge(s_x, 32 * (r + 1))
                nc.vector.tensor_tensor(
                    out=d0[:], in0=g0[:], in1=x0[:], op=Alu.subtract
                )
                vdrain()
                nc.vector.tensor_tensor(
                    out=d1[:], in0=g1[:], in1=x1[:], op=Alu.subtract
                )
                vdrain(f"V_EQD_r{r}")

                # result partials: m * sum(d^2)
                nc.vector.tensor_tensor(out=d2j[:], in0=d0[:], in1=d0[:], op=Alu.mult)
                vdrain()
                nc.vector.tensor_reduce(
                    out=r00[:], in_=d2j[:], axis=mybir.AxisListType.X, op=Alu.add
                )
                vdrain()
                nc.vector.tensor_tensor(out=rr0[:], in0=r00[:], in1=m0[:], op=Alu.mult)
                vdrain(f"V_RR0_r{r}")
                nc.vector.tensor_tensor(out=d2j[:], in0=d1[:], in1=d1[:], op=Alu.mult)
                vdrain()
                nc.vector.tensor_reduce(
                    out=r01[:], in_=d2j[:], axis=mybir.AxisListType.X, op=Alu.add
                )
                vdrain()
                nc.vector.tensor_tensor(out=rr1[:], in0=r01[:], in1=m1[:], op=Alu.mult)
                vdrain(f"V_RR1_r{r}")

                # counts and scales: scl = ALPHA / (cnt + 1)
                nc.vector.reduce_sum(c00[:], eq00[:], axis=mybir.AxisListType.X)
                vdrain()
                nc.vector.reduce_sum(c01[:], eq01[:], axis=mybir.AxisListType.X)
                vdrain()
                nc.vector.tensor_tensor(out=cnt0[:], in0=c00[:], in1=c01[:], op=Alu.add)
                vdrain()
                nc.vector.tensor_scalar_add(cnt0[:], cnt0[:], 1.0)
                vdrain()
                nc.vector.reciprocal(rec0[:], cnt0[:])
                vdrain()
                nc.vector.tensor_scalar_mul(scl0[:], rec0[:], ALPHA)
                vdrain()
                nc.vector.reduce_sum(c10[:], eq10[:], axis=mybir.AxisListType.X)
                vdrain()
                nc.vector.reduce_sum(c11[:], eq11[:], axis=mybir.AxisListType.X)
                vdrain()
                nc.vector.tensor_tensor(out=cnt1[:], in0=c10[:], in1=c11[:], op=Alu.add)
                vdrain()
                nc.vector.tensor_scalar_add(cnt1[:], cnt1[:], 1.0)
                vdrain()
                nc.vector.reciprocal(rec1[:], cnt1[:])
                vdrain()
                nc.vector.tensor_scalar_mul(scl1[:], rec1[:], ALPHA)
                vdrain()

                # updated rows: upd = g - scl * S
                nc.vector.wait_ge(s_pe, 4 * r + 3)
                nc.vector.tensor_scalar(
                    out=ssc0[:], in0=S0[:], scalar1=scl0[:, :1], scalar2=None,
                    op0=Alu.mult,
                )
                vdrain()
                nc.vector.tensor_tensor(
                    out=upd0[:], in0=g0[:], in1=ssc0[:], op=Alu.subtract
                )
                vdrain(f"V_UPD0_r{r}")
                nc.vector.wait_ge(s_pe, 4 * r + 4)
                nc.vector.tensor_scalar(
                    out=ssc1[:], in0=S1[:], scalar1=scl1[:, :1], scalar2=None,
                    op0=Alu.mult,
                )
                vdrain()
                nc.vector.tensor_tensor(
                    out=upd1[:], in0=g1[:], in1=ssc1[:], op=Alu.subtract
                )
                vdrain(f"V_UPD1_r{r}")

        @block.tensor
        def _(tensor):
            nc.tensor.wait_ge(s_gp, 2)
            for r in range(REPS):
                nc.tensor.wait_ge(s_v, vm[f"V_SKEY0_r{r}"])
                nc.tensor.transpose(
                    out=ps0[:], in_=sk0[:].to_broadcast([P, P]), identity=ident[:]
                ).then_inc(s_pe, 1)
                nc.tensor.wait_ge(s_v, vm[f"V_SKEY1_r{r}"])
                nc.tensor.transpose(
                    out=ps1[:], in_=sk1[:].to_broadcast([P, P]), identity=ident[:]
                ).then_inc(s_pe, 1)

                nc.tensor.wait_ge(s_v, vm[f"V_EQD_r{r}"])
                nc.tensor.matmul(
                    out=S0[:], lhsT=eq00[:], rhs=d0[:], start=True, stop=False
                )
                nc.tensor.matmul(
                    out=S0[:], lhsT=eq10[:], rhs=d1[:], start=False, stop=True
                ).then_inc(s_pe, 1)
                nc.tensor.matmul(
                    out=S1[:], lhsT=eq01[:], rhs=d0[:], start=True, stop=False
                )
                nc.tensor.matmul(
                    out=S1[:], lhsT=eq11[:], rhs=d1[:], start=False, stop=True
                ).then_inc(s_pe, 1)

        @block.gpsimd
        def _(gpsimd):
            nc.gpsimd.iota(
                iota[:],
                pattern=[[1, KC]],
                base=1,
                channel_multiplier=0,
                allow_small_or_imprecise_dtypes=True,
            ).then_inc(s_gp, 1)
            # identity matrix (inlined make_identity so we can chain the inc);
            # gpsimd ops can overlap across Q7 cores, so drain between the
            # memset and the affine_select that reads it.
            nc.gpsimd.memset(ident[:], 0.0)
            nc.gpsimd.drain()
            nc.gpsimd.affine_select(
                out=ident[:],
                in_=ident[:],
                compare_op=Alu.not_equal,
                fill=1.0,
                base=0,
                pattern=[[-1, P]],
                channel_multiplier=1,
            ).then_inc(s_gp, 1)

            bc_reg = nc.gpsimd.to_reg(KC - 1)
            for r in range(REPS):
                nc.gpsimd.wait_ge(s_v, vm[f"V_GIDX0_r{r}"])
                nc.gpsimd.indirect_dma_start(
                    out=g0[:], out_offset=None, in_=cen_t[:],
                    in_offset=bass.IndirectOffsetOnAxis(ap=gidx0[:, :1], axis=0),
                ).then_inc(s_g, 16)
                nc.gpsimd.wait_ge(s_v, vm[f"V_GIDX1_r{r}"])
                nc.gpsimd.indirect_dma_start(
                    out=g1[:], out_offset=None, in_=cen_t[:],
                    in_offset=bass.IndirectOffsetOnAxis(ap=gidx1[:, :1], axis=0),
                ).then_inc(s_g, 16)

                nc.gpsimd.wait_ge(s_cp, 16 * (r + 1))
                nc.gpsimd.wait_ge(s_v, vm[f"V_UPD0_r{r}"])
                nc.gpsimd.indirect_dma_start(
                    out=ncen_t[:],
                    out_offset=bass.IndirectOffsetOnAxis(ap=ski0[:, :1], axis=0),
                    in_=upd0[:], in_offset=None,
                    bounds_check=bc_reg, oob_is_err=False,
                ).then_inc(s_sc, 16)
                nc.gpsimd.wait_ge(s_v, vm[f"V_UPD1_r{r}"])
                nc.gpsimd.indirect_dma_start(
                    out=ncen_t[:],
                    out_offset=bass.IndirectOffsetOnAxis(ap=ski1[:, :1], axis=0),
                    in_=upd1[:], in_offset=None,
                    bounds_check=bc_reg, oob_is_err=False,
                ).then_inc(s_sc, 16)

        @block.scalar
        def _(scalar):
            # Bulk copy of the (untouched) centers shard, DRAM -> DRAM, on
            # the ACT HWDGE ring so it doesn't queue behind the SP loads.
            for r in range(REPS):
                if r > 0:
                    # prior rep's scatters must land before the copy rewrites
                    nc.scalar.wait_ge(s_sc, 32 * r)
                nc.scalar.dma_start(out=ncen_t[:], in_=cen_t[:]).then_inc(s_cp, 16)

        @block.sync
        def _(sync):
            for r in range(REPS):
                if r > 0:
                    # prior rep's vector stream is done with oh/x/rr buffers
                    nc.sync.wait_ge(s_v, vm[f"V_UPD1_r{r - 1}"])
                nc.sync.dma_start(out=oh0[:], in_=oh_t[0:P, :]).then_inc(s_oh, 16)
                nc.sync.dma_start(out=oh1[:], in_=oh_t[P:B, :]).then_inc(s_oh, 16)
                nc.sync.dma_start(out=x0[:], in_=x_t[0:P, :]).then_inc(s_x, 16)
                nc.sync.dma_start(out=x1[:], in_=x_t[P:B, :]).then_inc(s_x, 16)
                nc.sync.wait_ge(s_v, vm[f"V_RR0_r{r}"])
                nc.sync.dma_start(out=res_t[0:P, :], in_=rr0[:]).then_inc(s_res, 16)
                nc.sync.wait_ge(s_v, vm[f"V_RR1_r{r}"])
                nc.sync.dma_start(out=res_t[P:B, :], in_=rr1[:]).then_inc(s_res, 16)
            # Kernel-exit safety: everything observable has landed.
            nc.sync.wait_ge(s_sc, 32 * REPS)
            nc.sync.wait_ge(s_res, 32 * REPS)
            if debug_taps:
                nc.sync.wait_ge(s_v, vm["n"])
                nc.sync.wait_ge(s_pe, 4 * REPS)
                nc.sync.wait_ge(s_g, 32 * REPS)
                src = {
                    "t_lp0": lp0, "t_m0": m0, "t_gidx0": gidx0,
                    "t_sk0": sk0, "t_ski0": ski0, "t_cnt0": cnt0,
                    "t_scl0": scl0, "t_ident": ident, "t_skT0": skT0,
                    "t_eq00": eq00, "t_g0": g0, "t_d0": d0,
                    "t_ssc0": ssc0, "t_upd0": upd0,
                }
                for name, buf in src.items():
                    nc.sync.dma_start(
                        out=taps[name][:], in_=buf[:]
                    ).then_inc(s_res, 16)
                nc.sync.dma_start(
                    out=taps["t_iota"][:], in_=iota[:, 0:16]
                ).then_inc(s_res, 16)
                nc.sync.wait_ge(s_res, 32 * REPS + 16 * 15)

    return nc


def _get_nc():
    global _cached_nc
    if _cached_nc is None:
        _cached_nc = build_bass()
    return _cached_nc


def kernel(x, onehot, centers, _debug_results=None):
    x = np.ascontiguousarray(np.asarray(x, dtype=np.float32))
    onehot = np.asarray(onehot, dtype=np.float32)
    centers = np.asarray(centers, dtype=np.float32)

    oh_pad = np.zeros((B, CPAD), dtype=np.float32)
    oh_pad[:, :C] = onehot
    cen_pad = np.zeros((CPAD, D), dtype=np.float32)
    cen_pad[:C] = centers

    in_maps = []
    for i in range(NCORES):
        sl = slice(i * KC, (i + 1) * KC)
        in_maps.append({
            "x": x,
            "oh": np.ascontiguousarray(oh_pad[:, sl]),
            "cen": np.ascontiguousarray(cen_pad[sl]),
        })

    nc = _get_nc()
    br = run_bass_kernel_spmd(nc, in_maps, core_ids=list(range(NCORES)))
    outs = br.results
    if _debug_results is not None:
        _debug_results.append(br)

    new_centers = np.concatenate([o["ncen"] for o in outs], axis=0)[:C]
    result = np.zeros((B, 1), dtype=np.float32)
    for o in outs:
        result = result + o["res"]
    return result, new_centers
